# revision 1
# baseline (speedup 1.0000x reference)
"""DGCN (GCNConv + self/change terms) on 8 Trainium2 NeuronCores.

Strategy (dst-sharded graph parallelism):
  - Output nodes (segment-sum destinations) are sharded across the 8 cores;
    each core owns a contiguous range of 64-node "dst tiles".
  - Host sorts edges (incl. self-loops) by (dst tile, src), pads each tile's
    edge list to multiples of 128, and builds per-core tables:
      ix16[128, 8*B] int16 gather indices (dma_gather layout: flat edge i of
                          a call at [i%16, i//16], replicated to the 8
                          16-partition Q7 groups)
      dstl[128, B] f32    local dst (0..63) within the tile
      nrm[128, B]  f32    edge weight dinv[src]*dinv[dst] (0 for padding)
  - Device, per dst tile t: dma_gather of x[src] rows (up to 512 rows per
    call), build a one-hot matrix oh[e, dst] = (iota == dstl_e) * nrm_e on
    the vector engine, and accumulate zT[d, dst] += msgs_e^T @ oh on the
    tensor engine in PSUM. This performs the whole normalized scatter-add
    as matmuls.
  - dma_gather indices are int16, so the gather table is split in two DRAM
    tensors: x_full rows [0, 32768) and x_hi rows [32768, n_pad); each
    tile's (src-sorted) edges are split lo/hi at block granularity.
  - Algebraic folding: out = h_neigh + x@W0 + (h_neigh - x)@Wt
        = (z @ Wc + bc) @ (I + Wt) + x @ (W0 - Wt)
        = z @ C + x @ B2 + b'
    with C = Wc @ (I + Wt), B2 = W0 - Wt, b' = bc @ (I + Wt), and z the
    normalized neighbor sum (incl. self loops) of raw x rows. So the x@Wc
    matmul is applied *after* aggregation on 64-row tiles (8x less matmul
    work than computing x@Wc for all N on every core) and each core needs
    only two small constant weights.
"""

import numpy as np

N_NODES = 50000
D = 128
N_CORES = 8
TILE_DST = 64  # dst nodes per tile (matmul free dim)
BLK = 128  # edges per matmul block (PE contraction dim)
HALF = 32768  # int16 index limit -> gather table split point
CALL_BLKS = 8  # max blocks (128 idxs each) per dma_gather call (1024-idx HW cap)
N_SWDGE_QUEUES = 4  # parallel SWDGE descriptor-generation queues

_NC_CACHE = {}


def _host_prep(x, edge_index, Wc, bc, W0, Wt, n_cores=N_CORES, tile_dst=TILE_DST):
    n, d = x.shape
    src = np.asarray(edge_index[0], dtype=np.int64)
    dst = np.asarray(edge_index[1], dtype=np.int64)

    # in-degree incl. self loop -> symmetric normalization factors
    deg = (np.bincount(dst, minlength=n) + 1).astype(np.float32)
    dinv = (1.0 / np.sqrt(deg)).astype(np.float32)

    loops = np.arange(n, dtype=np.int64)
    src_a = np.concatenate([src, loops])
    dst_a = np.concatenate([dst, loops])
    norm_a = (dinv[src_a] * dinv[dst_a]).astype(np.float32)

    tiles_total = -(-n // tile_dst)
    tiles_total = -(-tiles_total // n_cores) * n_cores
    tpc = tiles_total // n_cores
    n_pad = tiles_total * tile_dst
    rows_pc = tpc * tile_dst

    tile_of = dst_a // tile_dst
    order = np.lexsort((src_a, tile_of))
    src_s = src_a[order]
    dstl_s = (dst_a[order] - tile_of[order] * tile_dst).astype(np.float32)
    norm_s = norm_a[order]
    tile_s = tile_of[order]

    half = HALF if n_pad > HALF else n_pad

    counts = np.bincount(tile_s, minlength=tiles_total)
    tile_starts = np.zeros(tiles_total + 1, np.int64)
    tile_starts[1:] = np.cumsum(counts)
    # per (core, tile): lo/hi split position (edges sorted by src)
    lo_counts = np.zeros(tiles_total, np.int64)
    for g in range(tiles_total):
        s0, c = tile_starts[g], counts[g]
        lo_counts[g] = np.searchsorted(src_s[s0 : s0 + c], half)
    hi_counts = counts - lo_counts

    def nblk(c):
        return -(-c // BLK)

    NB_lo = np.zeros(tpc, np.int64)
    NB_hi = np.zeros(tpc, np.int64)
    for i in range(tpc):
        g = np.arange(n_cores) * tpc + i
        NB_lo[i] = nblk(lo_counts[g]).max()
        NB_hi[i] = nblk(hi_counts[g]).max()
        if NB_lo[i] + NB_hi[i] == 0:
            NB_lo[i] = 1
    F = NB_lo + NB_hi  # blocks per tile slot
    B = int(F.sum())
    off = np.zeros(tpc, np.int64)
    off[1:] = np.cumsum(F)[:-1]

    idx_flat = np.zeros((n_cores, B * BLK), np.int32)  # per-edge gather index
    dst_t = np.zeros((n_cores, BLK, B), np.float32)
    nrm_t = np.zeros((n_cores, BLK, B), np.float32)
    for k in range(n_cores):
        for i in range(tpc):
            g = k * tpc + i
            s0 = int(tile_starts[g])
            clo, chi = int(lo_counts[g]), int(hi_counts[g])
            o = int(off[i])
            # lo edges -> blocks [o, o+NB_lo), hi -> [o+NB_lo, o+F)
            for (cnt, base_blk, idx_shift, pos) in (
                (clo, o, 0, s0),
                (chi, o + int(NB_lo[i]), half, s0 + clo),
            ):
                if cnt == 0:
                    continue
                nb = nblk(cnt)
                cap = nb * BLK
                bi = np.zeros(cap, np.int32)
                bd = np.zeros(cap, np.float32)
                bn = np.zeros(cap, np.float32)
                bi[:cnt] = src_s[pos : pos + cnt] - idx_shift
                bd[:cnt] = dstl_s[pos : pos + cnt]
                bn[:cnt] = norm_s[pos : pos + cnt]
                e0 = base_blk * BLK
                idx_flat[k][e0 : e0 + cap] = bi
                cols = slice(base_blk, base_blk + nb)
                dst_t[k][:, cols] = bd.reshape(nb, BLK).T
                nrm_t[k][:, cols] = bn.reshape(nb, BLK).T

    # dma_gather int16 index tensor: within a call (<= CALL_BLKS blocks),
    # flat edge i of the call sits at [i % 16, w0 + i // 16], replicated
    # across the eight 16-partition groups. Because calls are aligned to
    # block boundaries and a block is 128 = 8*16 edges, the global wrap
    # below produces exactly the per-call layout for any block range.
    ix16 = np.zeros((n_cores, BLK, B * (BLK // 16)), np.int16)
    for k in range(n_cores):
        v = idx_flat[k].astype(np.int16).reshape(B * (BLK // 16), 16).T
        for c in range(8):
            ix16[k][16 * c : 16 * (c + 1), :] = v

    # fused weights
    Wc64 = np.asarray(Wc, np.float64)
    Wt64 = np.asarray(Wt, np.float64)
    W064 = np.asarray(W0, np.float64)
    bc64 = np.asarray(bc, np.float64)
    B1 = np.eye(d) + Wt64
    C = (Wc64 @ B1).astype(np.float32)
    B2 = (W064 - Wt64).astype(np.float32)
    bp = (bc64 @ B1).astype(np.float32)

    x_pad = np.zeros((n_pad, d), np.float32)
    x_pad[:n] = np.asarray(x, np.float32)

    consts = {
        "cw": C,
        "b2w": B2,
        "bpb": np.broadcast_to(bp, (tile_dst, d)).copy(),
        "iota": np.broadcast_to(
            np.arange(tile_dst, dtype=np.float32), (BLK, tile_dst)
        ).copy(),
        "ident": np.eye(BLK, dtype=np.float32),
    }
    x_hi_arr = x_pad[half:] if n_pad > half else np.zeros((1, d), np.float32)
    in_maps = []
    for k in range(n_cores):
        m = dict(consts)
        m["x_full"] = x_pad[:half]
        m["x_hi"] = x_hi_arr
        m["x_own"] = x_pad[k * rows_pc : (k + 1) * rows_pc].copy()
        m["ix16"] = ix16[k]
        m["dst_t"] = dst_t[k]
        m["nrm_t"] = nrm_t[k]
        in_maps.append(m)

    meta = dict(
        F=F,
        NB_lo=NB_lo,
        NB_hi=NB_hi,
        off=off,
        B=B,
        tpc=tpc,
        n_pad=n_pad,
        rows_pc=rows_pc,
        d=d,
        half=half,
        hi_rows=x_hi_arr.shape[0],
    )
    return in_maps, meta


def _build_nc(meta, n_cores=N_CORES, tile_dst=TILE_DST, repeat=1, ablate=()):
    """ablate: subset of {"gather","onehot","segmm","epilogue","xown"} to
    drop from the program (timing bisection only — output becomes wrong)."""
    import contextlib

    import concourse.bacc as bacc
    import concourse.mybir as mybir
    import concourse.tile as tile
    from concourse import library_config

    f32 = mybir.dt.float32
    i16 = mybir.dt.int16
    F, NB_lo, NB_hi, off = meta["F"], meta["NB_lo"], meta["NB_hi"], meta["off"]
    B, tpc = meta["B"], meta["tpc"]
    n_pad, rows_pc, d = meta["n_pad"], meta["rows_pc"], meta["d"]
    W16 = B * (BLK // 16)

    nc = bacc.Bacc(
        "TRN2",
        target_bir_lowering=False,
        debug=False,
        num_devices=n_cores,
        num_swdge_queues=N_SWDGE_QUEUES,
    )
    x_full = nc.declare_dram_parameter("x_full", [meta["half"], d], f32, isOutput=False)
    x_hi = nc.declare_dram_parameter("x_hi", [meta["hi_rows"], d], f32, isOutput=False)
    x_own = nc.declare_dram_parameter("x_own", [rows_pc, d], f32, isOutput=False)
    ix16 = nc.declare_dram_parameter("ix16", [BLK, W16], i16, isOutput=False)
    dst_t = nc.declare_dram_parameter("dst_t", [BLK, B], f32, isOutput=False)
    nrm_t = nc.declare_dram_parameter("nrm_t", [BLK, B], f32, isOutput=False)
    cw = nc.declare_dram_parameter("cw", [d, d], f32, isOutput=False)
    b2w = nc.declare_dram_parameter("b2w", [d, d], f32, isOutput=False)
    bpb = nc.declare_dram_parameter("bpb", [tile_dst, d], f32, isOutput=False)
    iota = nc.declare_dram_parameter("iota", [BLK, tile_dst], f32, isOutput=False)
    ident = nc.declare_dram_parameter("ident", [BLK, BLK], f32, isOutput=False)
    out = nc.declare_dram_parameter("out", [rows_pc, d], f32, isOutput=True)

    eq, mul, add = (
        mybir.AluOpType.is_equal,
        mybir.AluOpType.mult,
        mybir.AluOpType.add,
    )

    with tile.TileContext(nc) as tc:
        with (
            tc.tile_pool(name="const", bufs=1) as cpool,
            tc.tile_pool(name="tbl", bufs=1) as tpool,
            tc.tile_pool(name="gather", bufs=3) as gpool,
            tc.tile_pool(name="work", bufs=3) as wpool,
            tc.tile_pool(name="oh", bufs=4) as ohpool,
            tc.tile_pool(name="zps", bufs=2, space="PSUM") as zpool,
            tc.tile_pool(name="tps", bufs=2, space="PSUM") as tpspool,
            tc.tile_pool(name="ops", bufs=2, space="PSUM") as opool,
        ):
            nc.gpsimd.load_library(library_config.mlp)
            c_sb = cpool.tile([d, d], f32)
            nc.sync.dma_start(out=c_sb[:], in_=cw[:])
            b2_sb = cpool.tile([d, d], f32)
            nc.sync.dma_start(out=b2_sb[:], in_=b2w[:])
            bp_sb = cpool.tile([tile_dst, d], f32)
            nc.sync.dma_start(out=bp_sb[:], in_=bpb[:])
            io_sb = cpool.tile([BLK, tile_dst], f32)
            nc.sync.dma_start(out=io_sb[:], in_=iota[:])
            id_sb = cpool.tile([BLK, BLK], f32)
            nc.sync.dma_start(out=id_sb[:], in_=ident[:])
            ix_sb = tpool.tile([BLK, W16], i16)
            nc.sync.dma_start(out=ix_sb[:], in_=ix16[:])
            dl_sb = tpool.tile([BLK, B], f32)
            nc.sync.dma_start(out=dl_sb[:], in_=dst_t[:])
            nm_sb = tpool.tile([BLK, B], f32)
            nc.sync.dma_start(out=nm_sb[:], in_=nrm_t[:])

            _q = [0]  # round-robin SWDGE queue assignment for gathers
            # repeat>1 wraps the whole body in a device-side loop; used only
            # by the timing harness to amplify device time vs host overhead.
            rep_ctx = tc.For_i(0, repeat, 1) if repeat > 1 else contextlib.nullcontext()
            with rep_ctx:
                for i in range(tpc):
                    fi = int(F[i])
                    o = int(off[i])
                    g = gpool.tile([BLK, fi * d], f32, tag="g")
                    if "gather" not in ablate:
                        for (tbl, blk0, nb_total) in (
                            (x_full, 0, int(NB_lo[i])),
                            (x_hi, int(NB_lo[i]), int(NB_hi[i])),
                        ):
                            for c in range(0, nb_total, CALL_BLKS):
                                nb = min(CALL_BLKS, nb_total - c)
                                col = blk0 + c
                                nidx = nb * BLK
                                nc.gpsimd.dma_gather(
                                    out_ap=g[:, col * d : (col + nb) * d].rearrange(
                                        "p (n e) -> p n e", e=d
                                    ),
                                    in_ap=tbl[:],
                                    idxs_ap=ix_sb[
                                        :, (o + col) * 8 : (o + col + nb) * 8
                                    ],
                                    num_idxs=nidx,
                                    num_idxs_reg=nidx,
                                    elem_size=d,
                                    queue_num=_q[0] % N_SWDGE_QUEUES,
                                )
                                _q[0] += 1
                    if "xown" not in ablate:
                        xo = wpool.tile([tile_dst, d], f32, tag="xo")
                        nc.sync.dma_start(
                            out=xo[:], in_=x_own[i * tile_dst : (i + 1) * tile_dst, :]
                        )
                        xt_ps = tpspool.tile([d, tile_dst], f32)
                        nc.tensor.transpose(
                            out=xt_ps[:],
                            in_=xo[:],
                            identity=id_sb[:tile_dst, :tile_dst],
                        )
                        xt_sb = wpool.tile([d, tile_dst], f32, tag="xt")
                        nc.scalar.copy(out=xt_sb[:], in_=xt_ps[:])

                    z_ps = zpool.tile([d, tile_dst], f32)
                    for j in range(fi):
                        if "onehot" not in ablate:
                            oh = ohpool.tile([BLK, tile_dst], f32, tag="oh")
                            nc.vector.tensor_scalar(
                                out=oh[:],
                                in0=io_sb[:],
                                scalar1=dl_sb[:, o + j : o + j + 1],
                                scalar2=nm_sb[:, o + j : o + j + 1],
                                op0=eq,
                                op1=mul,
                            )
                            rhs_mm = oh[:]
                        else:
                            rhs_mm = io_sb[:]
                        if "segmm" not in ablate:
                            nc.tensor.matmul(
                                out=z_ps[:],
                                lhsT=g[:, j * d : (j + 1) * d],
                                rhs=rhs_mm,
                                start=(j == 0),
                                stop=(j == fi - 1),
                            )
                    if "epilogue" not in ablate:
                        if "segmm" in ablate:
                            nc.vector.memset(z_ps[:], 0.0)
                        z_sb = wpool.tile([d, tile_dst], f32, tag="z")
                        nc.scalar.copy(out=z_sb[:], in_=z_ps[:])

                        o_ps = opool.tile([tile_dst, d], f32)
                        nc.tensor.matmul(
                            out=o_ps[:],
                            lhsT=z_sb[:],
                            rhs=c_sb[:],
                            start=True,
                            stop=False,
                        )
                        nc.tensor.matmul(
                            out=o_ps[:],
                            lhsT=xt_sb[:],
                            rhs=b2_sb[:],
                            start=False,
                            stop=True,
                        )
                        o_sb = wpool.tile([tile_dst, d], f32, tag="o")
                        nc.vector.tensor_tensor(
                            out=o_sb[:], in0=o_ps[:], in1=bp_sb[:], op=add
                        )
                        nc.sync.dma_start(
                            out=out[i * tile_dst : (i + 1) * tile_dst, :], in_=o_sb[:]
                        )
    nc.compile()
    return nc


def _get_nc(meta, n_cores=N_CORES, tile_dst=TILE_DST):
    key = (tuple(int(f) for f in meta["F"]), tuple(int(f) for f in meta["NB_lo"]))
    if key not in _NC_CACHE:
        _NC_CACHE[key] = _build_nc(meta, n_cores=n_cores, tile_dst=tile_dst)
    return _NC_CACHE[key]


_LAST_RESULTS = None


def kernel(x, edge_index, Wc, bc, W0, Wt):
    global _LAST_RESULTS
    from concourse.bass_utils import run_bass_kernel_spmd

    x = np.asarray(x)
    n = x.shape[0]
    in_maps, meta = _host_prep(x, edge_index, Wc, bc, W0, Wt)
    nc = _get_nc(meta)
    res = run_bass_kernel_spmd(nc, in_maps, list(range(N_CORES)))
    _LAST_RESULTS = res
    outs = [res.results[k]["out"] for k in range(N_CORES)]
    return np.concatenate(outs, axis=0)[:n].astype(np.float32)



# revision 4
# speedup vs baseline: 2.5978x; 2.5978x over previous
"""DGCN (GCNConv + self/change terms) on 8 Trainium2 NeuronCores.

Strategy (dst-sharded graph parallelism, v2):
  - Output nodes are processed in 64-node dst tiles. The 784 tiles are
    assigned to (core, slot) pairs by sorting tiles on their lo-edge count
    and giving 8 consecutive tiles (one per core) to each of the 98 slots;
    this equalizes per-slot segment lengths across cores so the single SPMD
    program wastes <4% padding.
  - Per core, edges (incl. self-loops) are laid out as two flat index
    streams (lo: src < 32768, hi: src >= 32768 — int16 gather-index limit),
    tile-major, each tile's segment padded only to the max count across the
    8 cores (edge granularity, no block alignment). dma_gather fetches the
    streams in 1024-index calls from x tables stored as bf16 rows packed
    into 64 f32 words (256B descriptors) — the packing halves the gather's
    per-index cost on the Pool engine, which is this kernel's bottleneck.
  - The normalized scatter-add z^T[d, dst] += msgs^T @ oh runs as bf16
    matmuls per 128-edge block. The one-hot blocks oh[e, dst] = nrm_e at
    column dstl_e are built on the HOST and streamed in as a bf16 tensor
    (DMA on the SP/Act HWDGE engines overlaps the Pool-engine gathers),
    so no vector-engine work is spent building them. Blocks straddling a
    tile boundary are consumed twice with complementary masks.
  - Algebraic folding (exact): out = z @ C + x @ B2 + b' with
    C = Wc @ (I + Wt), B2 = W0 - Wt, b' = bc @ (I + Wt).
    Epilogue per tile: o[64,128] = z_sb^T @ C + x_own^T^T @ B2 (+ bias),
    with x_own uploaded pre-transposed (bf16) and results batched 8 tiles
    per output store.
"""

import numpy as np
import ml_dtypes

N_NODES = 50000
D = 128
N_CORES = 8
TILE = 64  # dst nodes per tile
BLK = 128  # edges per matmul block
HALF = 32768  # int16 gather-index limit -> lo/hi table split
CALL = 1024  # gather indexes per dma_gather call
OHG = 32  # one-hot chunks per DMA group load
XTC = 16  # slots of x_own^T per DMA chunk
OWG = 8  # slots batched per output store
N_SWDGE_QUEUES = 4

BF16 = ml_dtypes.bfloat16

_NC_CACHE = {}


def _host_prep(x, edge_index, Wc, bc, W0, Wt):
    n, d = x.shape
    src = np.asarray(edge_index[0], dtype=np.int64)
    dst = np.asarray(edge_index[1], dtype=np.int64)

    deg = (np.bincount(dst, minlength=n) + 1).astype(np.float64)
    dinv = 1.0 / np.sqrt(deg)

    loops = np.arange(n, dtype=np.int64)
    s_a = np.concatenate([src, loops])
    d_a = np.concatenate([dst, loops])
    nrm_a = (dinv[s_a] * dinv[d_a]).astype(np.float32)

    tiles_total = -(-n // TILE)
    tiles_total = -(-tiles_total // N_CORES) * N_CORES
    tpc = tiles_total // N_CORES
    n_pad = tiles_total * TILE
    rows_pc = tpc * TILE

    tile_of = d_a // TILE
    dstl_a = (d_a % TILE).astype(np.int32)
    is_hi = s_a >= HALF

    lo_cnt = np.bincount(tile_of[~is_hi], minlength=tiles_total)
    hi_cnt = np.bincount(tile_of[is_hi], minlength=tiles_total)

    # balanced (core, slot) assignment: sort tiles by lo count, 8 consecutive
    # tiles -> one slot, one per core
    order = np.argsort(-lo_cnt, kind="stable")
    assign = order.reshape(tpc, N_CORES)  # [slot, core] -> tile id
    slot_of = np.empty(tiles_total, np.int64)
    core_of = np.empty(tiles_total, np.int64)
    slot_of[assign.ravel()] = np.repeat(np.arange(tpc), N_CORES)
    core_of[assign.ravel()] = np.tile(np.arange(N_CORES), tpc)

    L = lo_cnt[assign].max(axis=1)  # shared per-slot lo segment length
    H = hi_cnt[assign].max(axis=1)
    lo_off = np.zeros(tpc + 1, np.int64)
    lo_off[1:] = np.cumsum(L)
    hi_off = np.zeros(tpc + 1, np.int64)
    hi_off[1:] = np.cumsum(H)
    SL, SH = int(lo_off[-1]), int(hi_off[-1])
    SLP = -(-SL // BLK) * BLK
    SHP = -(-SH // BLK) * BLK
    NBL, NBH = SLP // BLK, SHP // BLK

    # consumption schedule: per slot, covering blocks of its lo then hi span
    schedule = []  # (kind, slot, block)
    slot_entries = []
    for i in range(tpc):
        ent = []
        if L[i] > 0:
            for b in range(int(lo_off[i]) // BLK, -(-int(lo_off[i] + L[i]) // BLK)):
                ent.append((0, b))
        if H[i] > 0:
            for b in range(int(hi_off[i]) // BLK, -(-int(hi_off[i] + H[i]) // BLK)):
                ent.append((1, b))
        slot_entries.append(ent)
        for kind, b in ent:
            schedule.append((kind, i, b))
    NCH = len(schedule)
    # chunk id lookup: (kind, slot, block) -> sequential id
    chunk_id = {(k, i, b): c for c, (k, i, b) in enumerate(schedule)}

    # per-edge stream positions (per core)
    core_e = core_of[tile_of]
    slot_e = slot_of[tile_of]
    # rank edges within (core, slot, half) groups
    sort_key = np.lexsort((s_a, is_hi, slot_e, core_e))
    cs = core_e[sort_key]
    sl = slot_e[sort_key]
    hf = is_hi[sort_key]
    grp = ((cs * tpc + sl) * 2 + hf).astype(np.int64)
    # rank within group
    first = np.ones(len(grp), bool)
    first[1:] = grp[1:] != grp[:-1]
    gstart = np.where(first)[0]
    rank = np.arange(len(grp)) - np.repeat(gstart, np.diff(np.append(gstart, len(grp))))
    seg_off = np.where(hf, hi_off[sl], lo_off[sl])
    pos = seg_off + rank  # stream position within (core, half-stream)
    srcv = s_a[sort_key]
    dstlv = dstl_a[sort_key]
    nrmv = nrm_a[sort_key]

    # fused weights (f64 for exactness, stored bf16)
    Wc64 = np.asarray(Wc, np.float64)
    Wt64 = np.asarray(Wt, np.float64)
    W064 = np.asarray(W0, np.float64)
    bc64 = np.asarray(bc, np.float64)
    B1 = np.eye(d) + Wt64
    C = (Wc64 @ B1).astype(BF16)
    B2 = (W064 - Wt64).astype(BF16)
    bp = (bc64 @ B1).astype(np.float32)

    x_pad = np.zeros((n_pad, d), np.float32)
    x_pad[:n] = np.asarray(x, np.float32)
    x_bf = x_pad.astype(BF16)
    y_pack = np.ascontiguousarray(x_bf).view(np.float32)  # [n_pad, 64]
    y_full = np.ascontiguousarray(y_pack[:HALF])
    y_hi = np.ascontiguousarray(y_pack[HALF:])

    consts = {
        "cw": np.ascontiguousarray(C),
        "b2w": np.ascontiguousarray(B2),
        "bpb": np.ascontiguousarray(np.broadcast_to(bp, (TILE, d))).astype(np.float32),
        "y_full": y_full,
        "y_hi": y_hi,
    }

    in_maps = []
    rowsels = []
    for c in range(N_CORES):
        m = dict(consts)
        sel = cs == c
        p = pos[sel]
        h = hf[sel]
        sv = srcv[sel]
        dv = dstlv[sel]
        nv = nrmv[sel]
        sle = sl[sel]

        idx_lo = np.zeros(SLP, np.int16)
        idx_hi = np.zeros(max(SHP, 16), np.int16)
        idx_lo[p[~h]] = sv[~h].astype(np.int16)
        idx_hi[p[h]] = (sv[h] - HALF).astype(np.int16)

        def wrap(v):
            w = v.reshape(-1, 16).T  # [16, len/16]
            return np.ascontiguousarray(np.tile(w, (8, 1)))

        m["ix_lo"] = wrap(idx_lo)
        m["ix_hi"] = wrap(idx_hi)

        # one-hot stream [NCH, 128, 64] -> [128, NCH*64] bf16
        blk = p // BLK
        cid = np.empty(len(p), np.int64)
        for k in range(2):
            mk = h == (k == 1)
            cid[mk] = [chunk_id[(k, int(s), int(b))] for s, b in zip(sle[mk], blk[mk])]
        A = np.zeros((NCH, BLK, TILE), np.float32)
        A[cid, p % BLK, dv] = nv
        m["ohs"] = np.ascontiguousarray(
            A.transpose(1, 0, 2).reshape(BLK, NCH * TILE)
        ).astype(BF16)

        # x_own^T (pre-transposed, bf16), slot order
        rowsel = (assign[:, c][:, None] * TILE + np.arange(TILE)[None, :]).ravel()
        m["xts"] = np.ascontiguousarray(x_bf[rowsel].T)
        rowsels.append(rowsel)
        in_maps.append(m)

    meta = dict(
        SLP=SLP, SHP=max(SHP, 16 * 16), NBL=NBL, NBH=NBH, NCH=NCH,
        tpc=tpc, n_pad=n_pad, rows_pc=rows_pc, d=d,
        hi_rows=n_pad - HALF,
        slot_entries=tuple(tuple(e) for e in slot_entries),
    )
    return in_maps, meta, rowsels


def _build_nc(meta):
    import concourse.bacc as bacc
    import concourse.mybir as mybir
    import concourse.tile as tile
    from concourse import library_config

    f32 = mybir.dt.float32
    bf16 = mybir.dt.bfloat16
    i16 = mybir.dt.int16
    add = mybir.AluOpType.add
    mult = mybir.AluOpType.mult

    SLP, SHP = meta["SLP"], meta["SHP"]
    NBL, NBH, NCH = meta["NBL"], meta["NBH"], meta["NCH"]
    tpc, rows_pc, d = meta["tpc"], meta["rows_pc"], meta["d"]
    slot_entries = meta["slot_entries"]
    PK = d // 2  # packed row width in f32 words

    nc = bacc.Bacc(
        "TRN2",
        target_bir_lowering=False,
        debug=False,
        num_devices=N_CORES,
        num_swdge_queues=N_SWDGE_QUEUES,
    )
    y_full = nc.declare_dram_parameter("y_full", [HALF, PK], f32, isOutput=False)
    y_hi = nc.declare_dram_parameter("y_hi", [meta["hi_rows"], PK], f32, isOutput=False)
    ix_lo = nc.declare_dram_parameter("ix_lo", [BLK, SLP // 16], i16, isOutput=False)
    ix_hi = nc.declare_dram_parameter("ix_hi", [BLK, SHP // 16], i16, isOutput=False)
    ohs = nc.declare_dram_parameter("ohs", [BLK, NCH * TILE], bf16, isOutput=False)
    xts = nc.declare_dram_parameter("xts", [d, rows_pc], bf16, isOutput=False)
    cw = nc.declare_dram_parameter("cw", [d, d], bf16, isOutput=False)
    b2w = nc.declare_dram_parameter("b2w", [d, d], bf16, isOutput=False)
    bpb = nc.declare_dram_parameter("bpb", [TILE, d], f32, isOutput=False)
    out = nc.declare_dram_parameter("out", [rows_pc, d], f32, isOutput=True)

    n_oh_groups = -(-NCH // OHG)
    n_xt_chunks = -(-tpc // XTC)

    with tile.TileContext(nc) as tc:
        with (
            tc.tile_pool(name="const", bufs=1) as cpool,
            tc.tile_pool(name="oh", bufs=3) as ohpool,
            tc.tile_pool(name="xt", bufs=2) as xtpool,
            tc.tile_pool(name="glo", bufs=4) as glopool,
            tc.tile_pool(name="ghi", bufs=4) as ghipool,
            tc.tile_pool(name="zps", bufs=2, space="PSUM") as zpool,
            tc.tile_pool(name="ops", bufs=4, space="PSUM") as opool,
            tc.tile_pool(name="zsb", bufs=3) as zsbpool,
            tc.tile_pool(name="ow", bufs=2) as owpool,
        ):
            nc.gpsimd.load_library(library_config.mlp)
            c_sb = cpool.tile([d, d], bf16)
            nc.scalar.dma_start(out=c_sb[:], in_=cw[:])
            b2_sb = cpool.tile([d, d], bf16)
            nc.scalar.dma_start(out=b2_sb[:], in_=b2w[:])
            bp_sb = cpool.tile([TILE, d], f32)
            nc.sync.dma_start(out=bp_sb[:], in_=bpb[:])
            ixlo_sb = cpool.tile([BLK, SLP // 16], i16)
            nc.sync.dma_start(out=ixlo_sb[:], in_=ix_lo[:])
            ixhi_sb = cpool.tile([BLK, SHP // 16], i16)
            nc.sync.dma_start(out=ixhi_sb[:], in_=ix_hi[:])

            gtiles = [{}, {}]  # kind -> call -> tile
            gspec = [
                (glopool, y_full, ixlo_sb, NBL, "glo"),
                (ghipool, y_hi, ixhi_sb, NBH, "ghi"),
            ]

            def ensure_gather(kind, call):
                if call in gtiles[kind]:
                    return
                pool, tbl, ix_sb, nb_total, tag = gspec[kind]
                nidx = min(CALL, nb_total * BLK - call * CALL)
                nblk = nidx // BLK
                g = pool.tile([BLK, nblk * PK], f32, tag=tag)
                nc.gpsimd.dma_gather(
                    out_ap=g[:].rearrange("p (n e) -> p n e", e=PK),
                    in_ap=tbl[:],
                    idxs_ap=ix_sb[:, call * (CALL // 16) : call * (CALL // 16) + nidx // 16],
                    num_idxs=nidx,
                    num_idxs_reg=nidx,
                    elem_size=PK,
                    queue_num=0,
                )
                gtiles[kind][call] = g

            ohgr = {}
            alt = [0]

            def ensure_oh(gid):
                if gid in ohgr:
                    return
                ncols = min(OHG, NCH - gid * OHG) * TILE
                t = ohpool.tile([BLK, ncols], bf16, tag="oh")
                eng = nc.sync if alt[0] % 2 == 0 else nc.scalar
                alt[0] += 1
                eng.dma_start(out=t[:], in_=ohs[:, gid * OHG * TILE : gid * OHG * TILE + ncols])
                ohgr[gid] = t

            xtch = {}

            def ensure_xt(ch):
                if ch in xtch:
                    return
                ncols = min(XTC * TILE, rows_pc - ch * XTC * TILE)
                t = xtpool.tile([d, ncols], bf16, tag="xt")
                eng = nc.scalar if alt[0] % 2 == 0 else nc.sync
                alt[0] += 1
                eng.dma_start(out=t[:], in_=xts[:, ch * XTC * TILE : ch * XTC * TILE + ncols])
                xtch[ch] = t

            cid = 0
            ow = None
            for i in range(tpc):
                ensure_xt(i // XTC)
                z_ps = zpool.tile([d, TILE], f32)
                ent = slot_entries[i]
                for k, (kind, b) in enumerate(ent):
                    ensure_gather(kind, b // (CALL // BLK))
                    g = gtiles[kind][b // (CALL // BLK)]
                    j = b % (CALL // BLK)
                    lhsT = g[:, j * PK : (j + 1) * PK].bitcast(bf16)
                    ensure_oh(cid // OHG)
                    oh_t = ohgr[cid // OHG]
                    rhs = oh_t[:, (cid % OHG) * TILE : (cid % OHG + 1) * TILE]
                    nc.tensor.matmul(
                        out=z_ps[:],
                        lhsT=lhsT,
                        rhs=rhs,
                        start=(k == 0),
                        stop=(k == len(ent) - 1),
                    )
                    cid += 1
                z_sb = zsbpool.tile([d, TILE], bf16, tag="z")
                if ent:
                    nc.vector.tensor_scalar(
                        out=z_sb[:], in0=z_ps[:], scalar1=1.0, scalar2=None, op0=mult
                    )
                else:
                    nc.vector.memset(z_sb[:], 0.0)

                xt = xtch[i // XTC]
                xo = (i % XTC) * TILE
                o_ps = opool.tile([TILE, d], f32)
                nc.tensor.matmul(out=o_ps[:], lhsT=z_sb[:], rhs=c_sb[:], start=True, stop=False)
                nc.tensor.matmul(
                    out=o_ps[:], lhsT=xt[:, xo : xo + TILE], rhs=b2_sb[:], start=False, stop=True
                )
                if i % OWG == 0:
                    ow = owpool.tile([TILE, OWG * d], f32, tag="ow")
                nc.vector.tensor_tensor(
                    out=ow[:, (i % OWG) * d : (i % OWG + 1) * d],
                    in0=o_ps[:],
                    in1=bp_sb[:],
                    op=add,
                )
                if i % OWG == OWG - 1 or i == tpc - 1:
                    nslots = i % OWG + 1
                    g0 = (i // OWG) * OWG
                    view = out[g0 * TILE : (g0 + nslots) * TILE, :].rearrange(
                        "(s l) f -> l s f", l=TILE
                    )
                    src_view = ow[:, : nslots * d].rearrange("l (s f) -> l s f", f=d)
                    eng = nc.sync if alt[0] % 2 == 0 else nc.scalar
                    alt[0] += 1
                    eng.dma_start(out=view, in_=src_view)
    nc.compile()
    return nc


def _get_nc(meta):
    key = (meta["SLP"], meta["SHP"], meta["NCH"], meta["slot_entries"])
    if key not in _NC_CACHE:
        _NC_CACHE[key] = _build_nc(meta)
    return _NC_CACHE[key]


_LAST_RESULTS = None


def kernel(x, edge_index, Wc, bc, W0, Wt):
    global _LAST_RESULTS
    from concourse.bass_utils import run_bass_kernel_spmd

    x = np.asarray(x)
    n = x.shape[0]
    in_maps, meta, rowsels = _host_prep(x, edge_index, Wc, bc, W0, Wt)
    nc = _get_nc(meta)
    res = run_bass_kernel_spmd(nc, in_maps, list(range(N_CORES)))
    _LAST_RESULTS = res
    full = np.zeros((meta["n_pad"], D), np.float32)
    for c in range(N_CORES):
        full[rowsels[c]] = res.results[c]["out"]
    return full[:n].astype(np.float32)


# revision 8
# speedup vs baseline: 2.9127x; 1.1212x over previous
"""DGCN (GCNConv + self/change terms) on 8 Trainium2 NeuronCores.

Strategy (dst-sharded graph parallelism, v2):
  - Output nodes are processed in 64-node dst tiles. The 784 tiles are
    assigned to (core, slot) pairs by sorting tiles on their lo-edge count
    and giving 8 consecutive tiles (one per core) to each of the 98 slots;
    this equalizes per-slot segment lengths across cores so the single SPMD
    program wastes <4% padding.
  - Per core, edges (incl. self-loops) are laid out as two flat index
    streams (lo: src < 32768, hi: src >= 32768 — int16 gather-index limit),
    tile-major, each tile's segment padded only to the max count across the
    8 cores (edge granularity, no block alignment). dma_gather fetches the
    streams in 1024-index calls from x tables stored as bf16 rows packed
    into 64 f32 words (256B descriptors) — the packing halves the gather's
    per-index cost on the Pool engine, which is this kernel's bottleneck.
  - The normalized scatter-add z^T[d, dst] += msgs^T @ oh runs as bf16
    matmuls per 128-edge block. The one-hot blocks oh[e, dst] = nrm_e at
    column dstl_e are built on the HOST and streamed in as a bf16 tensor
    (DMA on the SP/Act HWDGE engines overlaps the Pool-engine gathers),
    so no vector-engine work is spent building them. Blocks straddling a
    tile boundary are consumed twice with complementary masks.
  - Algebraic folding (exact): out = z @ C + x @ B2 + b' with
    C = Wc @ (I + Wt), B2 = W0 - Wt, b' = bc @ (I + Wt).
    Epilogue per tile: o[64,128] = z_sb^T @ C + x_own^T^T @ B2 (+ bias),
    with x_own uploaded pre-transposed (bf16) and results batched 8 tiles
    per output store.
"""

import numpy as np
import ml_dtypes

N_NODES = 50000
D = 128
N_CORES = 8
TILE = 64  # dst nodes per tile
BLK = 128  # edges per matmul block
HALF = 32768  # int16 gather-index limit -> lo/hi table split
CALL = 1024  # gather indexes per dma_gather call
OHG = 32  # one-hot chunks per DMA group load
XTC = 16  # slots of x_own^T per DMA chunk
OWG = 8  # slots batched per output store
N_SWDGE_QUEUES = 4

BF16 = ml_dtypes.bfloat16

_NC_CACHE = {}


def _host_prep(x, edge_index, Wc, bc, W0, Wt):
    n, d = x.shape
    src = np.asarray(edge_index[0], dtype=np.int64)
    dst = np.asarray(edge_index[1], dtype=np.int64)

    deg = (np.bincount(dst, minlength=n) + 1).astype(np.float64)
    dinv = 1.0 / np.sqrt(deg)

    loops = np.arange(n, dtype=np.int64)
    s_a = np.concatenate([src, loops])
    d_a = np.concatenate([dst, loops])
    nrm_a = (dinv[s_a] * dinv[d_a]).astype(np.float32)

    tiles_total = -(-n // TILE)
    tiles_total = -(-tiles_total // N_CORES) * N_CORES
    tpc = tiles_total // N_CORES
    n_pad = tiles_total * TILE
    rows_pc = tpc * TILE

    tile_of = d_a // TILE
    dstl_a = (d_a % TILE).astype(np.int32)
    is_hi = s_a >= HALF

    lo_cnt = np.bincount(tile_of[~is_hi], minlength=tiles_total)
    hi_cnt = np.bincount(tile_of[is_hi], minlength=tiles_total)

    # balanced (core, slot) assignment: sort tiles by lo count, 8 consecutive
    # tiles -> one slot, one per core
    order = np.argsort(-lo_cnt, kind="stable")
    assign = order.reshape(tpc, N_CORES)  # [slot, core] -> tile id
    slot_of = np.empty(tiles_total, np.int64)
    core_of = np.empty(tiles_total, np.int64)
    slot_of[assign.ravel()] = np.repeat(np.arange(tpc), N_CORES)
    core_of[assign.ravel()] = np.tile(np.arange(N_CORES), tpc)

    L = lo_cnt[assign].max(axis=1)  # shared per-slot lo segment length
    H = hi_cnt[assign].max(axis=1)
    lo_off = np.zeros(tpc + 1, np.int64)
    lo_off[1:] = np.cumsum(L)
    hi_off = np.zeros(tpc + 1, np.int64)
    hi_off[1:] = np.cumsum(H)
    SL, SH = int(lo_off[-1]), int(hi_off[-1])
    SLP = -(-SL // BLK) * BLK
    SHP = -(-SH // BLK) * BLK
    NBL, NBH = SLP // BLK, SHP // BLK

    # consumption schedule: per slot, covering blocks of its lo then hi span
    schedule = []  # (kind, slot, block)
    slot_entries = []
    for i in range(tpc):
        ent = []
        if L[i] > 0:
            for b in range(int(lo_off[i]) // BLK, -(-int(lo_off[i] + L[i]) // BLK)):
                ent.append((0, b))
        if H[i] > 0:
            for b in range(int(hi_off[i]) // BLK, -(-int(hi_off[i] + H[i]) // BLK)):
                ent.append((1, b))
        slot_entries.append(ent)
        for kind, b in ent:
            schedule.append((kind, i, b))
    NCH = len(schedule)
    # chunk id lookup: (kind, slot, block) -> sequential id
    chunk_id = {(k, i, b): c for c, (k, i, b) in enumerate(schedule)}

    # per-edge stream positions (per core)
    core_e = core_of[tile_of]
    slot_e = slot_of[tile_of]
    # rank edges within (core, slot, half) groups
    sort_key = np.lexsort((s_a, is_hi, slot_e, core_e))
    cs = core_e[sort_key]
    sl = slot_e[sort_key]
    hf = is_hi[sort_key]
    grp = ((cs * tpc + sl) * 2 + hf).astype(np.int64)
    # rank within group
    first = np.ones(len(grp), bool)
    first[1:] = grp[1:] != grp[:-1]
    gstart = np.where(first)[0]
    rank = np.arange(len(grp)) - np.repeat(gstart, np.diff(np.append(gstart, len(grp))))
    seg_off = np.where(hf, hi_off[sl], lo_off[sl])
    pos = seg_off + rank  # stream position within (core, half-stream)
    srcv = s_a[sort_key]
    dstlv = dstl_a[sort_key]
    nrmv = nrm_a[sort_key]

    # fused weights (f64 for exactness, stored bf16)
    Wc64 = np.asarray(Wc, np.float64)
    Wt64 = np.asarray(Wt, np.float64)
    W064 = np.asarray(W0, np.float64)
    bc64 = np.asarray(bc, np.float64)
    B1 = np.eye(d) + Wt64
    C = (Wc64 @ B1).astype(BF16)
    B2 = (W064 - Wt64).astype(BF16)
    bp = (bc64 @ B1).astype(np.float32)

    x_pad = np.zeros((n_pad, d), np.float32)
    x_pad[:n] = np.asarray(x, np.float32)
    x_bf = x_pad.astype(BF16)
    y_pack = np.ascontiguousarray(x_bf).view(np.float32)  # [n_pad, 64]
    y_full = np.ascontiguousarray(y_pack[:HALF])
    y_hi = np.ascontiguousarray(y_pack[HALF:])

    consts = {
        "cw": np.ascontiguousarray(C),
        "b2w": np.ascontiguousarray(B2),
        "bpw": np.ascontiguousarray(
            np.broadcast_to(bp, (TILE, OWG, d)).reshape(TILE, OWG * d)
        ).astype(np.float32),
        "y_full": y_full,
        "y_hi": y_hi,
    }

    in_maps = []
    rowsels = []
    for c in range(N_CORES):
        m = dict(consts)
        sel = cs == c
        p = pos[sel]
        h = hf[sel]
        sv = srcv[sel]
        dv = dstlv[sel]
        nv = nrmv[sel]
        sle = sl[sel]

        idx_lo = np.zeros(SLP, np.int16)
        idx_hi = np.zeros(max(SHP, 16), np.int16)
        idx_lo[p[~h]] = sv[~h].astype(np.int16)
        idx_hi[p[h]] = (sv[h] - HALF).astype(np.int16)

        def wrap(v):
            w = v.reshape(-1, 16).T  # [16, len/16]
            return np.ascontiguousarray(np.tile(w, (8, 1)))

        m["ix_lo"] = wrap(idx_lo)
        m["ix_hi"] = wrap(idx_hi)

        # one-hot stream [NCH, 128, 64] -> [128, NCH*64] bf16
        blk = p // BLK
        cid = np.empty(len(p), np.int64)
        for k in range(2):
            mk = h == (k == 1)
            cid[mk] = [chunk_id[(k, int(s), int(b))] for s, b in zip(sle[mk], blk[mk])]
        A = np.zeros((NCH, BLK, TILE), np.float32)
        A[cid, p % BLK, dv] = nv
        m["ohs"] = np.ascontiguousarray(
            A.transpose(1, 0, 2).reshape(BLK, NCH * TILE)
        ).astype(BF16)

        # x_own^T (pre-transposed, bf16), slot order
        rowsel = (assign[:, c][:, None] * TILE + np.arange(TILE)[None, :]).ravel()
        m["xts"] = np.ascontiguousarray(x_bf[rowsel].T)
        rowsels.append(rowsel)
        in_maps.append(m)

    meta = dict(
        SLP=SLP, SHP=max(SHP, 16 * 16), NBL=NBL, NBH=NBH, NCH=NCH,
        tpc=tpc, n_pad=n_pad, rows_pc=rows_pc, d=d,
        hi_rows=n_pad - HALF,
        slot_entries=tuple(tuple(e) for e in slot_entries),
    )
    return in_maps, meta, rowsels


def _build_nc(meta):
    import concourse.bacc as bacc
    import concourse.mybir as mybir
    import concourse.tile as tile
    from concourse import library_config

    f32 = mybir.dt.float32
    bf16 = mybir.dt.bfloat16
    i16 = mybir.dt.int16
    add = mybir.AluOpType.add
    mult = mybir.AluOpType.mult

    SLP, SHP = meta["SLP"], meta["SHP"]
    NBL, NBH, NCH = meta["NBL"], meta["NBH"], meta["NCH"]
    tpc, rows_pc, d = meta["tpc"], meta["rows_pc"], meta["d"]
    slot_entries = meta["slot_entries"]
    PK = d // 2  # packed row width in f32 words

    nc = bacc.Bacc(
        "TRN2",
        target_bir_lowering=False,
        debug=False,
        num_devices=N_CORES,
        num_swdge_queues=N_SWDGE_QUEUES,
    )
    y_full = nc.declare_dram_parameter("y_full", [HALF, PK], f32, isOutput=False)
    y_hi = nc.declare_dram_parameter("y_hi", [meta["hi_rows"], PK], f32, isOutput=False)
    ix_lo = nc.declare_dram_parameter("ix_lo", [BLK, SLP // 16], i16, isOutput=False)
    ix_hi = nc.declare_dram_parameter("ix_hi", [BLK, SHP // 16], i16, isOutput=False)
    ohs = nc.declare_dram_parameter("ohs", [BLK, NCH * TILE], bf16, isOutput=False)
    xts = nc.declare_dram_parameter("xts", [d, rows_pc], bf16, isOutput=False)
    cw = nc.declare_dram_parameter("cw", [d, d], bf16, isOutput=False)
    b2w = nc.declare_dram_parameter("b2w", [d, d], bf16, isOutput=False)
    bpw = nc.declare_dram_parameter("bpw", [TILE, OWG * d], f32, isOutput=False)
    out = nc.declare_dram_parameter("out", [rows_pc, d], bf16, isOutput=True)

    with tile.TileContext(nc) as tc:
        with (
            tc.tile_pool(name="const", bufs=1) as cpool,
            tc.tile_pool(name="oh", bufs=4) as ohpool,
            tc.tile_pool(name="xt", bufs=2) as xtpool,
            tc.tile_pool(name="glo", bufs=6) as glopool,
            tc.tile_pool(name="ghi", bufs=6) as ghipool,
            tc.tile_pool(name="zps", bufs=2, space="PSUM") as zpool,
            tc.tile_pool(name="ops", bufs=2, space="PSUM") as opool,
            tc.tile_pool(name="zsb", bufs=4) as zsbpool,
            tc.tile_pool(name="ow", bufs=2) as owpool,
        ):
            nc.gpsimd.load_library(library_config.mlp)
            c_sb = cpool.tile([d, d], bf16)
            nc.scalar.dma_start(out=c_sb[:], in_=cw[:])
            b2_sb = cpool.tile([d, d], bf16)
            nc.scalar.dma_start(out=b2_sb[:], in_=b2w[:])
            bp_sb = cpool.tile([TILE, OWG * d], f32)
            nc.sync.dma_start(out=bp_sb[:], in_=bpw[:])
            ixlo_sb = cpool.tile([BLK, SLP // 16], i16)
            nc.sync.dma_start(out=ixlo_sb[:], in_=ix_lo[:])
            ixhi_sb = cpool.tile([BLK, SHP // 16], i16)
            nc.scalar.dma_start(out=ixhi_sb[:], in_=ix_hi[:])

            gtiles = [{}, {}]  # kind -> call -> tile
            gspec = [
                (glopool, y_full, ixlo_sb, NBL, "glo"),
                (ghipool, y_hi, ixhi_sb, NBH, "ghi"),
            ]

            def ensure_gather(kind, call):
                if call in gtiles[kind]:
                    return
                pool, tbl, ix_sb, nb_total, tag = gspec[kind]
                nidx = min(CALL, nb_total * BLK - call * CALL)
                nblk = nidx // BLK
                g = pool.tile([BLK, nblk * PK], f32, tag=tag)
                nc.gpsimd.dma_gather(
                    out_ap=g[:].rearrange("p (n e) -> p n e", e=PK),
                    in_ap=tbl[:],
                    idxs_ap=ix_sb[:, call * (CALL // 16) : call * (CALL // 16) + nidx // 16],
                    num_idxs=nidx,
                    num_idxs_reg=nidx,
                    elem_size=PK,
                    queue_num=0,
                )
                gtiles[kind][call] = g

            ohgr = {}
            alt = [0]

            def ensure_oh(gid):
                if gid in ohgr:
                    return
                ncols = min(OHG, NCH - gid * OHG) * TILE
                t = ohpool.tile([BLK, ncols], bf16, tag="oh")
                eng = nc.sync if alt[0] % 2 == 0 else nc.scalar
                alt[0] += 1
                eng.dma_start(out=t[:], in_=ohs[:, gid * OHG * TILE : gid * OHG * TILE + ncols])
                ohgr[gid] = t

            xtch = {}

            def ensure_xt(ch):
                if ch in xtch:
                    return
                ncols = min(XTC * TILE, rows_pc - ch * XTC * TILE)
                t = xtpool.tile([d, ncols], bf16, tag="xt")
                eng = nc.scalar if alt[0] % 2 == 0 else nc.sync
                alt[0] += 1
                eng.dma_start(out=t[:], in_=xts[:, ch * XTC * TILE : ch * XTC * TILE + ncols])
                xtch[ch] = t

            state = {"o_ps": None, "cid": 0, "z_ps": None, "zsb": None}
            pending = []  # (slot, z_sb tile, column offset) awaiting epilogue

            def issue_epilogue(j, z_sb, zoff):
                xt = xtch[j // XTC]
                xo = (j % XTC) * TILE
                if j % OWG == 0:
                    o_ps_new = opool.tile([TILE, OWG * d], f32, tag="op")
                    state["o_ps"] = o_ps_new
                o_ps = state["o_ps"]
                oc = (j % OWG) * d
                nc.tensor.matmul(
                    out=o_ps[:, oc : oc + d],
                    lhsT=z_sb[:, zoff : zoff + TILE],
                    rhs=c_sb[:],
                    start=True,
                    stop=False,
                )
                nc.tensor.matmul(
                    out=o_ps[:, oc : oc + d],
                    lhsT=xt[:, xo : xo + TILE],
                    rhs=b2_sb[:],
                    start=False,
                    stop=True,
                )
                if j % OWG == OWG - 1 or j == tpc - 1:
                    nslots = j % OWG + 1
                    ow = owpool.tile([TILE, nslots * d], bf16, tag="ow")
                    nc.vector.tensor_tensor(
                        out=ow[:],
                        in0=o_ps[:, : nslots * d],
                        in1=bp_sb[:, : nslots * d],
                        op=add,
                    )
                    g0 = (j // OWG) * OWG
                    view = out[g0 * TILE : (g0 + nslots) * TILE, :].rearrange(
                        "(s l) f -> l s f", l=TILE
                    )
                    src_view = ow[:].rearrange("l (s f) -> l s f", f=d)
                    eng = nc.sync if alt[0] % 2 == 0 else nc.scalar
                    alt[0] += 1
                    eng.dma_start(out=view, in_=src_view)

            for i in range(tpc):
                ensure_xt(i // XTC)
                if i % 2 == 0:
                    z_ps_new = zpool.tile([d, 2 * TILE], f32, tag="zp")
                    state["z_ps"] = z_ps_new
                z_ps = state["z_ps"]
                zo = (i % 2) * TILE
                ent = slot_entries[i]
                for k, (kind, b) in enumerate(ent):
                    ensure_gather(kind, b // (CALL // BLK))
                    g = gtiles[kind][b // (CALL // BLK)]
                    j = b % (CALL // BLK)
                    lhsT = g[:, j * PK : (j + 1) * PK].bitcast(bf16)
                    cid = state["cid"]
                    ensure_oh(cid // OHG)
                    oh_t = ohgr[cid // OHG]
                    rhs = oh_t[:, (cid % OHG) * TILE : (cid % OHG + 1) * TILE]
                    nc.tensor.matmul(
                        out=z_ps[:, zo : zo + TILE],
                        lhsT=lhsT,
                        rhs=rhs,
                        start=(k == 0),
                        stop=(k == len(ent) - 1),
                    )
                    state["cid"] += 1
                if i % 2 == 1 or i == tpc - 1:
                    npair = (i % 2) + 1
                    z_sb = zsbpool.tile([d, npair * TILE], bf16, tag="z")
                    nc.vector.tensor_scalar(
                        out=z_sb[:],
                        in0=z_ps[:, : npair * TILE],
                        scalar1=1.0,
                        scalar2=None,
                        op0=mult,
                    )
                    for q in range(npair):
                        pending.append((i - npair + 1 + q, z_sb, q * TILE))
                while pending and pending[0][0] <= i - 2:
                    issue_epilogue(*pending.pop(0))
            while pending:
                issue_epilogue(*pending.pop(0))
    nc.compile()
    return nc


def _get_nc(meta):
    key = (meta["SLP"], meta["SHP"], meta["NCH"], meta["slot_entries"])
    if key not in _NC_CACHE:
        _NC_CACHE[key] = _build_nc(meta)
    return _NC_CACHE[key]


_LAST_RESULTS = None


def kernel(x, edge_index, Wc, bc, W0, Wt):
    global _LAST_RESULTS
    from concourse.bass_utils import run_bass_kernel_spmd

    x = np.asarray(x)
    n = x.shape[0]
    in_maps, meta, rowsels = _host_prep(x, edge_index, Wc, bc, W0, Wt)
    nc = _get_nc(meta)
    res = run_bass_kernel_spmd(nc, in_maps, list(range(N_CORES)))
    _LAST_RESULTS = res
    full = np.zeros((meta["n_pad"], D), np.float32)
    for c in range(N_CORES):
        full[rowsels[c]] = res.results[c]["out"]
    return full[:n].astype(np.float32)


# revision 9
# speedup vs baseline: 2.9412x; 1.0098x over previous
"""DGCN (GCNConv + self/change terms) on 8 Trainium2 NeuronCores.

Strategy (dst-sharded graph parallelism, v2):
  - Output nodes are processed in 64-node dst tiles. The 784 tiles are
    assigned to (core, slot) pairs by sorting tiles on their lo-edge count
    and giving 8 consecutive tiles (one per core) to each of the 98 slots;
    this equalizes per-slot segment lengths across cores so the single SPMD
    program wastes <4% padding.
  - Per core, edges (incl. self-loops) are laid out as two flat index
    streams (lo: src < 32768, hi: src >= 32768 — int16 gather-index limit),
    tile-major, each tile's segment padded only to the max count across the
    8 cores (edge granularity, no block alignment). dma_gather fetches the
    streams in 1024-index calls from x tables stored as bf16 rows packed
    into 64 f32 words (256B descriptors) — the packing halves the gather's
    per-index cost on the Pool engine, which is this kernel's bottleneck.
  - The normalized scatter-add z^T[d, dst] += msgs^T @ oh runs as bf16
    matmuls per 128-edge block. The one-hot blocks oh[e, dst] = nrm_e at
    column dstl_e are built on the HOST and streamed in as a bf16 tensor
    (DMA on the SP/Act HWDGE engines overlaps the Pool-engine gathers),
    so no vector-engine work is spent building them. Blocks straddling a
    tile boundary are consumed twice with complementary masks.
  - Algebraic folding (exact): out = z @ C + x @ B2 + b' with
    C = Wc @ (I + Wt), B2 = W0 - Wt, b' = bc @ (I + Wt).
    Epilogue per tile: o[64,128] = z_sb^T @ C + x_own^T^T @ B2 (+ bias),
    with x_own uploaded pre-transposed (bf16) and results batched 8 tiles
    per output store.
"""

import numpy as np
import ml_dtypes

N_NODES = 50000
D = 128
N_CORES = 8
TILE = 64  # dst nodes per tile
BLK = 128  # edges per matmul block
HALF = 32768  # int16 gather-index limit -> lo/hi table split
CALL = 1024  # gather indexes per dma_gather call
OHG = 16  # one-hot chunks per DMA group load
XTC = 16  # slots of x_own^T per DMA chunk
OWG = 8  # slots batched per output store
N_SWDGE_QUEUES = 4

BF16 = ml_dtypes.bfloat16

_NC_CACHE = {}


def _host_prep(x, edge_index, Wc, bc, W0, Wt):
    n, d = x.shape
    src = np.asarray(edge_index[0], dtype=np.int64)
    dst = np.asarray(edge_index[1], dtype=np.int64)

    deg = (np.bincount(dst, minlength=n) + 1).astype(np.float64)
    dinv = 1.0 / np.sqrt(deg)

    loops = np.arange(n, dtype=np.int64)
    s_a = np.concatenate([src, loops])
    d_a = np.concatenate([dst, loops])
    nrm_a = (dinv[s_a] * dinv[d_a]).astype(np.float32)

    tiles_total = -(-n // TILE)
    tiles_total = -(-tiles_total // N_CORES) * N_CORES
    tpc = tiles_total // N_CORES
    n_pad = tiles_total * TILE
    rows_pc = tpc * TILE

    tile_of = d_a // TILE
    dstl_a = (d_a % TILE).astype(np.int32)
    is_hi = s_a >= HALF

    lo_cnt = np.bincount(tile_of[~is_hi], minlength=tiles_total)
    hi_cnt = np.bincount(tile_of[is_hi], minlength=tiles_total)

    # balanced (core, slot) assignment: sort tiles by lo count, 8 consecutive
    # tiles -> one slot, one per core
    order = np.argsort(-lo_cnt, kind="stable")
    assign = order.reshape(tpc, N_CORES)  # [slot, core] -> tile id
    slot_of = np.empty(tiles_total, np.int64)
    core_of = np.empty(tiles_total, np.int64)
    slot_of[assign.ravel()] = np.repeat(np.arange(tpc), N_CORES)
    core_of[assign.ravel()] = np.tile(np.arange(N_CORES), tpc)

    L = lo_cnt[assign].max(axis=1)  # shared per-slot lo segment length
    H = hi_cnt[assign].max(axis=1)
    lo_off = np.zeros(tpc + 1, np.int64)
    lo_off[1:] = np.cumsum(L)
    hi_off = np.zeros(tpc + 1, np.int64)
    hi_off[1:] = np.cumsum(H)
    SL, SH = int(lo_off[-1]), int(hi_off[-1])
    SLP = -(-SL // BLK) * BLK
    SHP = -(-SH // BLK) * BLK
    NBL, NBH = SLP // BLK, SHP // BLK

    # consumption schedule: per slot, covering blocks of its lo then hi span
    schedule = []  # (kind, slot, block)
    slot_entries = []
    for i in range(tpc):
        ent = []
        if L[i] > 0:
            for b in range(int(lo_off[i]) // BLK, -(-int(lo_off[i] + L[i]) // BLK)):
                ent.append((0, b))
        if H[i] > 0:
            for b in range(int(hi_off[i]) // BLK, -(-int(hi_off[i] + H[i]) // BLK)):
                ent.append((1, b))
        slot_entries.append(ent)
        for kind, b in ent:
            schedule.append((kind, i, b))
    NCH = len(schedule)
    # chunk id lookup: (kind, slot, block) -> sequential id
    chunk_id = {(k, i, b): c for c, (k, i, b) in enumerate(schedule)}

    # per-edge stream positions (per core)
    core_e = core_of[tile_of]
    slot_e = slot_of[tile_of]
    # rank edges within (core, slot, half) groups
    sort_key = np.lexsort((s_a, is_hi, slot_e, core_e))
    cs = core_e[sort_key]
    sl = slot_e[sort_key]
    hf = is_hi[sort_key]
    grp = ((cs * tpc + sl) * 2 + hf).astype(np.int64)
    # rank within group
    first = np.ones(len(grp), bool)
    first[1:] = grp[1:] != grp[:-1]
    gstart = np.where(first)[0]
    rank = np.arange(len(grp)) - np.repeat(gstart, np.diff(np.append(gstart, len(grp))))
    seg_off = np.where(hf, hi_off[sl], lo_off[sl])
    pos = seg_off + rank  # stream position within (core, half-stream)
    srcv = s_a[sort_key]
    dstlv = dstl_a[sort_key]
    nrmv = nrm_a[sort_key]

    # fused weights (f64 for exactness, stored bf16)
    Wc64 = np.asarray(Wc, np.float64)
    Wt64 = np.asarray(Wt, np.float64)
    W064 = np.asarray(W0, np.float64)
    bc64 = np.asarray(bc, np.float64)
    B1 = np.eye(d) + Wt64
    C = (Wc64 @ B1).astype(BF16)
    B2 = (W064 - Wt64).astype(BF16)
    bp = (bc64 @ B1).astype(np.float32)

    x_pad = np.zeros((n_pad, d), np.float32)
    x_pad[:n] = np.asarray(x, np.float32)
    x_bf = x_pad.astype(BF16)
    y_pack = np.ascontiguousarray(x_bf).view(np.float32)  # [n_pad, 64]
    y_full = np.ascontiguousarray(y_pack[:HALF])
    y_hi = np.ascontiguousarray(y_pack[HALF:])

    consts = {
        "cw": np.ascontiguousarray(C),
        "b2w": np.ascontiguousarray(B2),
        "bpw": np.ascontiguousarray(
            np.broadcast_to(bp, (TILE, OWG, d)).reshape(TILE, OWG * d)
        ).astype(np.float32),
        "y_full": y_full,
        "y_hi": y_hi,
    }

    in_maps = []
    rowsels = []
    for c in range(N_CORES):
        m = dict(consts)
        sel = cs == c
        p = pos[sel]
        h = hf[sel]
        sv = srcv[sel]
        dv = dstlv[sel]
        nv = nrmv[sel]
        sle = sl[sel]

        idx_lo = np.zeros(SLP, np.int16)
        idx_hi = np.zeros(max(SHP, 16), np.int16)
        idx_lo[p[~h]] = sv[~h].astype(np.int16)
        idx_hi[p[h]] = (sv[h] - HALF).astype(np.int16)

        def wrap(v):
            w = v.reshape(-1, 16).T  # [16, len/16]
            return np.ascontiguousarray(np.tile(w, (8, 1)))

        m["ix_lo"] = wrap(idx_lo)
        m["ix_hi"] = wrap(idx_hi)

        # one-hot stream [NCH, 128, 64] -> [128, NCH*64] bf16
        blk = p // BLK
        cid = np.empty(len(p), np.int64)
        for k in range(2):
            mk = h == (k == 1)
            cid[mk] = [chunk_id[(k, int(s), int(b))] for s, b in zip(sle[mk], blk[mk])]
        A = np.zeros((NCH, BLK, TILE), np.float32)
        A[cid, p % BLK, dv] = nv
        m["ohs"] = np.ascontiguousarray(
            A.transpose(1, 0, 2).reshape(BLK, NCH * TILE)
        ).astype(BF16)

        # x_own^T (pre-transposed, bf16), slot order
        rowsel = (assign[:, c][:, None] * TILE + np.arange(TILE)[None, :]).ravel()
        m["xts"] = np.ascontiguousarray(x_bf[rowsel].T)
        rowsels.append(rowsel)
        in_maps.append(m)

    meta = dict(
        SLP=SLP, SHP=max(SHP, 16 * 16), NBL=NBL, NBH=NBH, NCH=NCH,
        tpc=tpc, n_pad=n_pad, rows_pc=rows_pc, d=d,
        hi_rows=n_pad - HALF,
        slot_entries=tuple(tuple(e) for e in slot_entries),
    )
    return in_maps, meta, rowsels


def _build_nc(meta):
    import concourse.bacc as bacc
    import concourse.mybir as mybir
    import concourse.tile as tile
    from concourse import library_config

    f32 = mybir.dt.float32
    bf16 = mybir.dt.bfloat16
    i16 = mybir.dt.int16
    add = mybir.AluOpType.add
    mult = mybir.AluOpType.mult

    SLP, SHP = meta["SLP"], meta["SHP"]
    NBL, NBH, NCH = meta["NBL"], meta["NBH"], meta["NCH"]
    tpc, rows_pc, d = meta["tpc"], meta["rows_pc"], meta["d"]
    slot_entries = meta["slot_entries"]
    PK = d // 2  # packed row width in f32 words

    nc = bacc.Bacc(
        "TRN2",
        target_bir_lowering=False,
        debug=False,
        num_devices=N_CORES,
        num_swdge_queues=N_SWDGE_QUEUES,
    )
    y_full = nc.declare_dram_parameter("y_full", [HALF, PK], f32, isOutput=False)
    y_hi = nc.declare_dram_parameter("y_hi", [meta["hi_rows"], PK], f32, isOutput=False)
    ix_lo = nc.declare_dram_parameter("ix_lo", [BLK, SLP // 16], i16, isOutput=False)
    ix_hi = nc.declare_dram_parameter("ix_hi", [BLK, SHP // 16], i16, isOutput=False)
    ohs = nc.declare_dram_parameter("ohs", [BLK, NCH * TILE], bf16, isOutput=False)
    xts = nc.declare_dram_parameter("xts", [d, rows_pc], bf16, isOutput=False)
    cw = nc.declare_dram_parameter("cw", [d, d], bf16, isOutput=False)
    b2w = nc.declare_dram_parameter("b2w", [d, d], bf16, isOutput=False)
    bpw = nc.declare_dram_parameter("bpw", [TILE, OWG * d], f32, isOutput=False)
    out = nc.declare_dram_parameter("out", [rows_pc, d], bf16, isOutput=True)

    with tile.TileContext(nc) as tc:
        with (
            tc.tile_pool(name="const", bufs=1) as cpool,
            tc.tile_pool(name="oh", bufs=6) as ohpool,
            tc.tile_pool(name="xt", bufs=2) as xtpool,
            tc.tile_pool(name="glo", bufs=8) as glopool,
            tc.tile_pool(name="ghi", bufs=8) as ghipool,
            tc.tile_pool(name="zps", bufs=2, space="PSUM") as zpool,
            tc.tile_pool(name="ops", bufs=3, space="PSUM") as opool,
            tc.tile_pool(name="zsb", bufs=6) as zsbpool,
            tc.tile_pool(name="ow", bufs=2) as owpool,
        ):
            nc.gpsimd.load_library(library_config.mlp)
            c_sb = cpool.tile([d, d], bf16)
            nc.scalar.dma_start(out=c_sb[:], in_=cw[:])
            b2_sb = cpool.tile([d, d], bf16)
            nc.scalar.dma_start(out=b2_sb[:], in_=b2w[:])
            bp_sb = cpool.tile([TILE, OWG * d], f32)
            nc.sync.dma_start(out=bp_sb[:], in_=bpw[:])
            ixlo_sb = cpool.tile([BLK, SLP // 16], i16)
            nc.sync.dma_start(out=ixlo_sb[:], in_=ix_lo[:])
            ixhi_sb = cpool.tile([BLK, SHP // 16], i16)
            nc.scalar.dma_start(out=ixhi_sb[:], in_=ix_hi[:])

            gtiles = [{}, {}]  # kind -> call -> tile
            gspec = [
                (glopool, y_full, ixlo_sb, NBL, "glo"),
                (ghipool, y_hi, ixhi_sb, NBH, "ghi"),
            ]

            def ensure_gather(kind, call):
                if call in gtiles[kind]:
                    return
                pool, tbl, ix_sb, nb_total, tag = gspec[kind]
                nidx = min(CALL, nb_total * BLK - call * CALL)
                nblk = nidx // BLK
                g = pool.tile([BLK, nblk * PK], f32, tag=tag)
                nc.gpsimd.dma_gather(
                    out_ap=g[:].rearrange("p (n e) -> p n e", e=PK),
                    in_ap=tbl[:],
                    idxs_ap=ix_sb[:, call * (CALL // 16) : call * (CALL // 16) + nidx // 16],
                    num_idxs=nidx,
                    num_idxs_reg=nidx,
                    elem_size=PK,
                    queue_num=0,
                )
                gtiles[kind][call] = g

            ohgr = {}
            alt = [0]

            def ensure_oh(gid):
                if gid in ohgr:
                    return
                ncols = min(OHG, NCH - gid * OHG) * TILE
                t = ohpool.tile([BLK, ncols], bf16, tag="oh")
                eng = nc.sync if alt[0] % 2 == 0 else nc.scalar
                alt[0] += 1
                eng.dma_start(out=t[:], in_=ohs[:, gid * OHG * TILE : gid * OHG * TILE + ncols])
                ohgr[gid] = t

            xtch = {}

            def ensure_xt(ch):
                if ch in xtch:
                    return
                ncols = min(XTC * TILE, rows_pc - ch * XTC * TILE)
                t = xtpool.tile([d, ncols], bf16, tag="xt")
                eng = nc.scalar if alt[0] % 2 == 0 else nc.sync
                alt[0] += 1
                eng.dma_start(out=t[:], in_=xts[:, ch * XTC * TILE : ch * XTC * TILE + ncols])
                xtch[ch] = t

            state = {"o_ps": None, "cid": 0, "z_ps": None, "zsb": None}
            pending = []  # (slot, z_sb tile, column offset) awaiting epilogue

            def issue_epilogue(j, z_sb, zoff):
                xt = xtch[j // XTC]
                xo = (j % XTC) * TILE
                if j % OWG == 0:
                    o_ps_new = opool.tile([TILE, OWG * d], f32, tag="op")
                    state["o_ps"] = o_ps_new
                o_ps = state["o_ps"]
                oc = (j % OWG) * d
                nc.tensor.matmul(
                    out=o_ps[:, oc : oc + d],
                    lhsT=z_sb[:, zoff : zoff + TILE],
                    rhs=c_sb[:],
                    start=True,
                    stop=False,
                )
                nc.tensor.matmul(
                    out=o_ps[:, oc : oc + d],
                    lhsT=xt[:, xo : xo + TILE],
                    rhs=b2_sb[:],
                    start=False,
                    stop=True,
                )
                if j % OWG == OWG - 1 or j == tpc - 1:
                    nslots = j % OWG + 1
                    ow = owpool.tile([TILE, nslots * d], bf16, tag="ow")
                    nc.vector.tensor_tensor(
                        out=ow[:],
                        in0=o_ps[:, : nslots * d],
                        in1=bp_sb[:, : nslots * d],
                        op=add,
                    )
                    g0 = (j // OWG) * OWG
                    view = out[g0 * TILE : (g0 + nslots) * TILE, :].rearrange(
                        "(s l) f -> l s f", l=TILE
                    )
                    src_view = ow[:].rearrange("l (s f) -> l s f", f=d)
                    eng = nc.sync if alt[0] % 2 == 0 else nc.scalar
                    alt[0] += 1
                    eng.dma_start(out=view, in_=src_view)

            for i in range(tpc):
                ensure_xt(i // XTC)
                if (i // XTC + 1) * XTC * TILE < rows_pc:
                    ensure_xt(i // XTC + 1)
                if i % 2 == 0:
                    z_ps_new = zpool.tile([d, 2 * TILE], f32, tag="zp")
                    state["z_ps"] = z_ps_new
                z_ps = state["z_ps"]
                zo = (i % 2) * TILE
                ent = slot_entries[i]
                for k, (kind, b) in enumerate(ent):
                    call = b // (CALL // BLK)
                    ensure_gather(kind, call)
                    if (call + 1) * CALL < gspec[kind][3] * BLK:
                        ensure_gather(kind, call + 1)
                    g = gtiles[kind][call]
                    j = b % (CALL // BLK)
                    lhsT = g[:, j * PK : (j + 1) * PK].bitcast(bf16)
                    cid = state["cid"]
                    ensure_oh(cid // OHG)
                    if (cid // OHG + 1) * OHG < NCH:
                        ensure_oh(cid // OHG + 1)
                    oh_t = ohgr[cid // OHG]
                    rhs = oh_t[:, (cid % OHG) * TILE : (cid % OHG + 1) * TILE]
                    nc.tensor.matmul(
                        out=z_ps[:, zo : zo + TILE],
                        lhsT=lhsT,
                        rhs=rhs,
                        start=(k == 0),
                        stop=(k == len(ent) - 1),
                    )
                    state["cid"] += 1
                if i % 2 == 1 or i == tpc - 1:
                    npair = (i % 2) + 1
                    z_sb = zsbpool.tile([d, npair * TILE], bf16, tag="z")
                    nc.vector.tensor_scalar(
                        out=z_sb[:],
                        in0=z_ps[:, : npair * TILE],
                        scalar1=1.0,
                        scalar2=None,
                        op0=mult,
                    )
                    for q in range(npair):
                        pending.append((i - npair + 1 + q, z_sb, q * TILE))
                while pending and pending[0][0] <= i - 2:
                    issue_epilogue(*pending.pop(0))
            while pending:
                issue_epilogue(*pending.pop(0))
    nc.compile()
    return nc


def _get_nc(meta):
    key = (meta["SLP"], meta["SHP"], meta["NCH"], meta["slot_entries"])
    if key not in _NC_CACHE:
        _NC_CACHE[key] = _build_nc(meta)
    return _NC_CACHE[key]


_LAST_RESULTS = None


def kernel(x, edge_index, Wc, bc, W0, Wt):
    global _LAST_RESULTS
    from concourse.bass_utils import run_bass_kernel_spmd

    x = np.asarray(x)
    n = x.shape[0]
    in_maps, meta, rowsels = _host_prep(x, edge_index, Wc, bc, W0, Wt)
    nc = _get_nc(meta)
    res = run_bass_kernel_spmd(nc, in_maps, list(range(N_CORES)))
    _LAST_RESULTS = res
    full = np.zeros((meta["n_pad"], D), np.float32)
    for c in range(N_CORES):
        full[rowsels[c]] = res.results[c]["out"]
    return full[:n].astype(np.float32)
